# revision 19
# baseline (speedup 1.0000x reference)
"""Binary conv forward kernel for Trainium2 (8 NeuronCores, data-parallel over batch).

Computes y = conv2d(sign(x), scale[o] * sign(w)), stride 1, pad 1, NCHW/OIHW,
x [16, 64, 224, 224] f32, w [64*64*3*3, 1] f32 -> y [16, 64, 224, 224] f32.

Sharding: batch 16 -> 2 images per core, weights replicated (tiny).

HBM traffic is the roofline, so both directions are compressed:
  - Input: the host ships only the HIGH BYTE of each f32 (pure strided
    relayout, no arithmetic), packed [128, pair, batch, w].  Reinterpreted as
    fp8e4, that byte has the same sign as x (sign bit + top-7 exponent bits),
    so ScalarE's sign() recovers sign(x) exactly -- randn never produces
    |x| < 2^-126 (byte 0x00/0x80) or |x| >= 2^127 (NaN bytes).  4x fewer
    input bytes.
  - Output: written as fp16.  y = scale[o] * S with S an integer in
    [-576, 576]; fp16 rounding error is <= 2^-11 relative, far inside the
    2e-2 gate.  2x fewer output bytes.  Host upcasts to f32 on unpack.

Device algorithm (per core, n_batch=2 images):
  - A resident fp8 sign plane [128, NV+1, 464]: slot j = image rows (2j, 2j+1)
    (even row on partitions 0..63, odd on 64..127), both batch images in the
    free dim with one shared zero column between and zero pads at the edges so
    the kw shifts are exact.  Slot NV = copy of slot 0 (for the row-0/223
    boundary unit).  ScalarE signs 8 rows per ACTIVATE to amortize overhead.
  - Interior output pair (2m+1, 2m+2) accumulates in one PSUM bank via 3
    DoubleRow matmuls (virtual K=256 over slots m, m+1; M=128; N=450), one per
    kw shift.  Stationary blocks [[W0,0],[W1,W0]] / [[W2,W1],[0,W2]], where
    Wk = sign(w)[:,:,kh=k,kw]^T.  Boundary rows 0 and 223 use slots NV-1 and
    NV with blocks [[0,W0],[0,W1]] / [[W1,0],[W2,0]].
  - PSUM tiles span 4 banks = 4 units; eviction multiplies by the
    per-partition scale[o] (computed on device from raw weights) and writes
    fp16: one VectorE op per 4 units (last 3 groups ride ScalarE to balance
    engine load).
  - Input DMAs ride the HWDGE ring (nc.sync); weight + output DMAs ride SWDGE
    (nc.gpsimd) so loads and stores overlap on HBM.
"""

import numpy as np
import ml_dtypes

import concourse.bacc as bacc
import concourse.mybir as mybir
import concourse.tile as tile

F32 = mybir.dt.float32
F16 = mybir.dt.float16
FP8 = mybir.dt.float8e4

N_CORES = 8
FULL_BATCH = 16
C = 64  # in channels == out channels
H = 224
W = 224
KH = KW = 3
NV = H // 2  # row-pair slots / output units per image pair
# Sign-plane slot layout (fp8, per partition): [pad, b0 w=224, sep, b1 w=224,
# pad, pad] = 452 cols used, padded to 464 (multiple of 16 for DoubleRow AP
# steps).
SW = 464   # slot stride
SN = 449   # matmul N (448 real output columns + 1 junk)

GU = 2           # units per PSUM tile (2 banks; bufs=4 -> PE runs 3 groups ahead)
OG = 8           # units per output chunk
ACT_EVICT_TAIL = (53, 55)  # post-sign-stream evictions moved to ScalarE
DVE_SIGN0 = False  # ScalarE chain [sign0, sblk] gates MM0 earlier than DVE bitwise
N_WARMUP_MM = 16  # dummy matmuls on a zeroed tile bridge engine-init to the first sign
SWI = False


def build_nc(n_batch=2):
    """Build the single-core Bass module (same NEFF runs on all 8 cores)."""
    nc = bacc.Bacc("TRN2", target_bir_lowering=False, debug=False)

    w = W
    xb = nc.dram_tensor("xb", [128, NV, n_batch, w], FP8, kind="ExternalInput")
    wraw = nc.dram_tensor("wraw", [C * C * KH * KW, 1], F32, kind="ExternalInput")
    # wblk: host-arranged raw f32 weights in the 6-tile DoubleRow block layout
    # [128, (3 interior + 3 boundary) tiles, 2, 128] with zeros in the zero
    # blocks (pure replication/padding; sign runs on device, sign(0) = 0).
    wblk = nc.dram_tensor("wblk", [128, 6 * 2 * 128], FP8, kind="ExternalInput")
    yb = nc.dram_tensor("yb", [128, NV, n_batch, w], F16, kind="ExternalOutput")

    xr = xb.ap().rearrange("p j b w -> p j (b w)")   # [128, NV, 448]
    yr = yb.ap().rearrange("p j b w -> p j (b w)")

    # Input chunk schedule: small leading chunks so signing (and the first
    # matmuls) start as early as possible, then full chunks.
    sizes = [4, 4, 8] + [16] * 6
    assert sum(sizes) == NV
    starts = list(np.cumsum([0] + sizes[:-1]))

    # PSUM groups: 55 x 2 interior units, then [110, boundary].
    n_groups = 56

    with tile.TileContext(nc) as tc:
        with (
            tc.tile_pool(name="wpool", bufs=1) as wpool,
            tc.tile_pool(name="icpool", bufs=4) as icpool,
            tc.tile_pool(name="pspool", bufs=4, space="PSUM") as pspool,
            tc.tile_pool(name="ocpool", bufs=4) as ocpool,
        ):
            # Warmup tile: N_WARMUP_MM dummy matmuls on zeroed SBUF ramp
            # the PE clock out of its low p-state while the first input chunk
            # and weights are still in flight.
            zt = wpool.tile([128, 2, SW], FP8)
            nc.vector.memset(zt[:], 0.0)

            # Weights land as the f32 high bytes (fp8 view): same sign, zero
            # blocks stay zero, and the gating DMA is 4x smaller.
            wblkf = wpool.tile([128, 6, 2, 128], FP8)
            sblk = wpool.tile([128, 6, 2, 128], FP8)
            wdr = [sblk[:, kw, :, :] for kw in range(3)]
            wb = [sblk[:, 3 + kw, :, :] for kw in range(3)]

            # scale[o] = mean(|w[o]|), O on partitions, duplicated on both
            # partition halves for the [128]-row eviction.
            w2 = wpool.tile([128, 576], F32)
            absw = wpool.tile([128, 576], F32)
            sc_sum = wpool.tile([128, 1], F32)
            sc128 = wpool.tile([128, 1], F32)

            # Resident sign plane; slot NV = V_0 copy.  Zero the pad columns
            # once (plane slots are written exactly once).
            assert n_batch == 2
            plane = wpool.tile([128, NV + 1, SW], FP8)
            nc.vector.memset(plane[:, :, 0:1], 0.0)
            nc.vector.memset(plane[:, :, 225:226], 0.0)
            nc.vector.memset(plane[:, :, 450:452], 0.0)

            def rhs(j, kw):
                return plane[:, j : j + 2, kw : kw + SN]

            def emit_sign(ic, c0, r0, k):
                """Sign rows r0..r0+k (absolute) from chunk starting at c0.
                ScalarE uses the Sign activation; GpSimd computes the same
                result bitwise on the raw bytes: (b & 0x80) | 0x38 is fp8
                +/-1.0 with the sign bit of b (x is never exactly 0)."""
                out_ap = plane[:, r0 : r0 + k, 1:451].rearrange(
                    "p j (b w) -> p j b w", w=225
                )[:, :, :, 0:w]
                in_ap = ic[:, r0 - c0 : r0 - c0 + k, :].rearrange(
                    "p j (b w) -> p j b w", b=n_batch
                )
                if DVE_SIGN0 and r0 < 8:
                    # First two batches ride VectorE so the first matmuls are
                    # not gated behind ScalarE's table load + weight sign.
                    nc.vector.tensor_scalar(
                        out_ap.bitcast(mybir.dt.uint8),
                        in_ap.bitcast(mybir.dt.uint8),
                        0x80, 0x38,
                        mybir.AluOpType.bitwise_and,
                        mybir.AluOpType.bitwise_or,
                    )
                else:
                    nc.scalar.sign(out_ap, in_ap)

            def evict(engine, ps, oc, oslot):
                out_ap = oc[:, oslot : oslot + GU, :].rearrange(
                    "p u (b w) -> p u b w", b=n_batch
                )
                in_ap = ps[:, 0:GU, 0:450].rearrange(
                    "p u (b w) -> p u b w", w=225
                )[:, :, :, 0:w]
                if engine == "act":
                    nc.scalar.mul(out_ap, in_ap, sc128[:])
                else:
                    nc.vector.tensor_scalar_mul(out_ap, in_ap, sc128[:])

            pm = (
                mybir.MatmulPerfMode.DoubleRowSwInterleave
                if SWI
                else mybir.MatmulPerfMode.DoubleRow
            )

            # ---- main pipeline ----
            next_chunk = 0   # next input chunk to DMA
            rows_avail = 0   # rows resident in SBUF (DMA emitted)
            rows_signed = 0  # rows signed into the plane
            ic = None
            ic_c0 = 0

            def ensure_signed(upto):
                """Emit chunk DMAs + sign ops until rows [0, upto) signed."""
                nonlocal next_chunk, rows_avail, rows_signed, ic, ic_c0
                while rows_signed < upto:
                    if rows_signed == rows_avail:
                        gc = sizes[next_chunk]
                        c0 = starts[next_chunk]
                        ic = icpool.tile([128, 16, n_batch * w], FP8, tag="ic")
                        nc.sync.dma_start(ic[:, 0:gc, :], xr[:, c0 : c0 + gc, :])
                        ic_c0 = c0
                        rows_avail += gc
                        next_chunk += 1
                    k = min(8, rows_avail - rows_signed)
                    emit_sign(ic, ic_c0, rows_signed, k)
                    rows_signed += k
                    if rows_signed - k == 0:
                        # V_0 copy for the boundary unit right after the
                        # first sign batch; then the weight-block sign (the
                        # first matmul's other gate) and the scale input.
                        nc.vector.tensor_copy(
                            out=plane[:, NV, 0:452], in_=plane[:, 0, 0:452]
                        )
                        nc.gpsimd.dma_start(
                            wblkf[:],
                            wblk.ap().rearrange("p (t i m) -> p t i m", t=6, i=2),
                        )
                        nc.scalar.sign(sblk[:], wblkf[:])
                        wr = wraw.ap().rearrange("(o f) one -> o (f one)", o=C)
                        nc.sync.dma_start(w2[0:64], wr)
                        nc.sync.dma_start(w2[64:128], wr)
                        # scale prep must be emitted before the first
                        # eviction (which reads sc128); it gates nothing
                        # earlier than that.
                        nc.scalar.activation(
                            out=absw[:], in_=w2[:],
                            func=mybir.ActivationFunctionType.Abs,
                            accum_out=sc_sum[:],
                        )
                        nc.scalar.mul(sc128[:], sc_sum[:], 1.0 / 576.0)

            # PE warmup: dep-free dummy matmuls on the zeroed tile.
            if N_WARMUP_MM:
                psw = pspool.tile([128, GU, 512], F32, tag="ps")
                for _ in range(N_WARMUP_MM):
                    nc.tensor.matmul(
                        psw[:, 0, 0:SN], zt[:, :, 0:128], zt[:, 0:2, 0:SN],
                        start=True, stop=True, perf_mode=pm,
                    )

            oc = None
            for g in range(n_groups):
                m0 = GU * g
                boundary = g == n_groups - 1
                nu = GU - 1 if boundary else GU  # interior units in group
                ensure_signed(min(m0 + nu + 1, NV))

                if m0 % OG == 0:
                    oc = ocpool.tile([128, OG, n_batch * w], F16, tag="oc")

                ps = pspool.tile([128, GU, 512], F32, tag="ps")
                for u in range(nu):
                    m = m0 + u
                    for kw in range(3):
                        nc.tensor.matmul(
                            ps[:, u, 0:SN], wdr[kw][:], rhs(m, kw),
                            start=(kw == 0), stop=(kw == 2),
                            perf_mode=pm,
                        )
                if boundary:
                    for kw in range(3):
                        nc.tensor.matmul(
                            ps[:, GU - 1, 0:SN], wb[kw][:], rhs(NV - 1, kw),
                            start=(kw == 0), stop=(kw == 2),
                            perf_mode=pm,
                        )

                evict("act" if g in ACT_EVICT_TAIL else "dve", ps, oc, m0 % OG)

                base = (m0 // OG) * OG
                if base == NV - OG:
                    # tail: store in halves so the last DMA is short
                    if m0 % OG == OG // 2 - GU:
                        nc.gpsimd.dma_start(
                            yr[:, base : base + OG // 2, :], oc[:, 0 : OG // 2, :]
                        )
                    elif boundary:
                        nc.gpsimd.dma_start(
                            yr[:, base + OG // 2 : base + OG, :],
                            oc[:, OG // 2 : OG, :],
                        )
                elif m0 % OG == OG - GU:
                    nc.gpsimd.dma_start(
                        yr[:, base : base + OG, :], oc[:, 0:OG, :]
                    )

    nc.compile()
    return nc


_NC_CACHE = {}


def _get_nc(key=(2,)):
    if key not in _NC_CACHE:
        _NC_CACHE[key] = build_nc(*key)
    return _NC_CACHE[key]


def _make_wblk(weights):
    """Arrange raw f32 weights into the 6-tile DoubleRow block layout
    [128, 6, 2, 128] (pure replication/zero-padding; sign runs on device)."""
    wt = weights.reshape(C, C, KH, KW).transpose(1, 2, 3, 0)  # [i, kh, kw, o]

    def T(kh, kw):
        return wt[:, kh, kw, :]  # W_{kh,kw}^T as [i, o]

    blk = np.zeros((128, 6, 2, 128), np.float32)
    for kw in range(KW):
        # interior tiles: i=0 -> [[W0, 0], [W1, W0]], i=1 -> [[W2, W1], [0, W2]]
        blk[0:64, kw, 0, 0:64] = T(0, kw)
        blk[64:128, kw, 0, 0:64] = T(1, kw)
        blk[64:128, kw, 0, 64:128] = T(0, kw)
        blk[0:64, kw, 1, 0:64] = T(2, kw)
        blk[0:64, kw, 1, 64:128] = T(1, kw)
        blk[64:128, kw, 1, 64:128] = T(2, kw)
        # boundary tiles: i=0 -> [[0, W0], [0, W1]], i=1 -> [[W1, 0], [W2, 0]]
        blk[0:64, 3 + kw, 0, 64:128] = T(0, kw)
        blk[64:128, 3 + kw, 0, 64:128] = T(1, kw)
        blk[0:64, 3 + kw, 1, 0:64] = T(1, kw)
        blk[64:128, 3 + kw, 1, 0:64] = T(2, kw)
    if SWI:
        # DoubleRowSwInterleave layout: per stationary tile, pairs (A, B)
        # interleaved per output column, columns reversed.
        swi = np.zeros((128, 6, 256), np.float32)
        swi[:, :, 0::2] = blk[:, :, 0, ::-1]
        swi[:, :, 1::2] = blk[:, :, 1, ::-1]
        blk = swi.reshape(128, 6, 2, 128)
    # ship only the f32 high byte (same sign; zeros stay 0x00 = fp8 zero)
    hb = blk.reshape(128, 6 * 2 * 128, 1).view(np.uint8)[:, :, 3]
    return np.ascontiguousarray(hb).view(ml_dtypes.float8_e4m3fn)


def pack_x(x_shard):
    """f32 [nb, C, h, w] -> high-byte plane [128, NV, nb, w] (fp8e4 view);
    p = parity*64 + channel.  Pure strided relayout of the sign/exponent
    byte -- no arithmetic."""
    nb = x_shard.shape[0]
    hb = x_shard.view(np.uint8).reshape(nb, C, NV, 2, W, 4)[..., 3]
    packed = np.ascontiguousarray(hb.transpose(3, 1, 2, 0, 4)).reshape(
        128, NV, nb, W
    )
    return packed.view(ml_dtypes.float8_e4m3fn)


def unpack_y(ypk):
    """fp16 [128, NV, nb, w] -> f32 [nb, C, h, w] per the unit layout."""
    nb = ypk.shape[2]
    y = np.empty((nb, C, H, W), np.float32)
    # interior units m=0..NV-2 -> rows 2m+1 (p<64) and 2m+2 (p>=64)
    y[:, :, 1 : H - 1 : 2] = ypk[0:C, 0 : NV - 1].transpose(2, 0, 1, 3)
    y[:, :, 2 : H : 2] = ypk[C:128, 0 : NV - 1].transpose(2, 0, 1, 3)
    # boundary unit: p<64 -> row 0, p>=64 -> row H-1
    y[:, :, 0] = ypk[0:C, NV - 1].transpose(1, 0, 2)
    y[:, :, H - 1] = ypk[C:128, NV - 1].transpose(1, 0, 2)
    return y


def make_in_maps(x, weights):
    x = np.ascontiguousarray(np.asarray(x, dtype=np.float32))
    weights = np.asarray(weights, dtype=np.float32)
    wblk = _make_wblk(weights)
    nb = FULL_BATCH // N_CORES
    return [
        {
            "xb": pack_x(x[c * nb : (c + 1) * nb]),
            "wraw": weights,
            "wblk": wblk,
        }
        for c in range(N_CORES)
    ]


def gather_out(results):
    return np.concatenate([unpack_y(r["yb"]) for r in results], axis=0)


def kernel(x, weights):
    from concourse import bass_utils

    nc = _get_nc()
    in_maps = make_in_maps(x, weights)
    res = bass_utils.run_bass_kernel_spmd(nc, in_maps, core_ids=list(range(N_CORES)))
    return gather_out(res.results)


# revision 20
# speedup vs baseline: 1.0265x; 1.0265x over previous
"""Binary conv forward kernel for Trainium2 (8 NeuronCores, data-parallel over batch).

Computes y = conv2d(sign(x), scale[o] * sign(w)), stride 1, pad 1, NCHW/OIHW,
x [16, 64, 224, 224] f32, w [64*64*3*3, 1] f32 -> y [16, 64, 224, 224] f32.

Sharding: batch 16 -> 2 images per core, weights replicated (tiny).

HBM traffic is the roofline, so both directions are compressed:
  - Input: the host ships only the HIGH BYTE of each f32 (pure strided
    relayout, no arithmetic), packed [128, pair, batch, w].  Reinterpreted as
    fp8e4, that byte has the same sign as x (sign bit + top-7 exponent bits),
    so ScalarE's sign() recovers sign(x) exactly -- randn never produces
    |x| < 2^-126 (byte 0x00/0x80) or |x| >= 2^127 (NaN bytes).  4x fewer
    input bytes.
  - Output: written as fp16.  y = scale[o] * S with S an integer in
    [-576, 576]; fp16 rounding error is <= 2^-11 relative, far inside the
    2e-2 gate.  2x fewer output bytes.  Host upcasts to f32 on unpack.

Device algorithm (per core, n_batch=2 images):
  - A resident fp8 sign plane [128, NV+1, 464]: slot j = image rows (2j, 2j+1)
    (even row on partitions 0..63, odd on 64..127), both batch images in the
    free dim with one shared zero column between and zero pads at the edges so
    the kw shifts are exact.  Slot NV = copy of slot 0 (for the row-0/223
    boundary unit).  ScalarE signs 8 rows per ACTIVATE to amortize overhead.
  - Interior output pair (2m+1, 2m+2) accumulates in one PSUM bank via 3
    DoubleRow matmuls (virtual K=256 over slots m, m+1; M=128; N=450), one per
    kw shift.  Stationary blocks [[W0,0],[W1,W0]] / [[W2,W1],[0,W2]], where
    Wk = sign(w)[:,:,kh=k,kw]^T.  Boundary rows 0 and 223 use slots NV-1 and
    NV with blocks [[0,W0],[0,W1]] / [[W1,0],[W2,0]].
  - PSUM tiles span 4 banks = 4 units; eviction multiplies by the
    per-partition scale[o] (computed on device from raw weights) and writes
    fp16: one VectorE op per 4 units (last 3 groups ride ScalarE to balance
    engine load).
  - Input DMAs ride the HWDGE ring (nc.sync); weight + output DMAs ride SWDGE
    (nc.gpsimd) so loads and stores overlap on HBM.
"""

import numpy as np
import ml_dtypes

import concourse.bacc as bacc
import concourse.mybir as mybir
import concourse.tile as tile

F32 = mybir.dt.float32
F16 = mybir.dt.float16
FP8 = mybir.dt.float8e4

N_CORES = 8
FULL_BATCH = 16
C = 64  # in channels == out channels
H = 224
W = 224
KH = KW = 3
NV = H // 2  # row-pair slots / output units per image pair
# Sign-plane slot layout (fp8, per partition): [pad, b0 w=224, sep, b1 w=224,
# pad, pad] = 452 cols used, padded to 464 (multiple of 16 for DoubleRow AP
# steps).
SW = 464   # slot stride
SN = 449   # matmul N (448 real output columns + 1 junk)

GU = 2           # units per PSUM tile (2 banks; bufs=4 -> PE runs 3 groups ahead)
OG = 8           # units per output chunk
ACT_EVICT_TAIL = (53, 55)  # post-sign-stream evictions moved to ScalarE
DVE_SIGN0 = True  # first two sign batches on VectorE (bitwise)
N_WARMUP_MM = 16  # dummy matmuls on a zeroed tile bridge engine-init to the first sign
SWI = False


def build_nc(n_batch=2):
    """Build the single-core Bass module (same NEFF runs on all 8 cores)."""
    nc = bacc.Bacc("TRN2", target_bir_lowering=False, debug=False)

    w = W
    xb = nc.dram_tensor("xb", [128, NV, n_batch, w], FP8, kind="ExternalInput")
    wraw = nc.dram_tensor("wraw", [C * C * KH * KW, 1], F32, kind="ExternalInput")
    # wblk: host-arranged raw f32 weights in the 6-tile DoubleRow block layout
    # [128, (3 interior + 3 boundary) tiles, 2, 128] with zeros in the zero
    # blocks (pure replication/padding; sign runs on device, sign(0) = 0).
    wblk = nc.dram_tensor("wblk", [128, 6 * 2 * 128], FP8, kind="ExternalInput")
    yb = nc.dram_tensor("yb", [128, NV, n_batch, w], F16, kind="ExternalOutput")

    xr = xb.ap().rearrange("p j b w -> p j (b w)")   # [128, NV, 448]
    yr = yb.ap().rearrange("p j b w -> p j (b w)")

    # Input chunk schedule: small leading chunks so signing (and the first
    # matmuls) start as early as possible, then full chunks.
    sizes = [4, 4, 8] + [16] * 6
    assert sum(sizes) == NV
    starts = list(np.cumsum([0] + sizes[:-1]))

    # PSUM groups: 55 x 2 interior units, then [110, boundary].
    n_groups = 56

    with tile.TileContext(nc) as tc:
        with (
            tc.tile_pool(name="wpool", bufs=1) as wpool,
            tc.tile_pool(name="icpool", bufs=4) as icpool,
            tc.tile_pool(name="pspool", bufs=4, space="PSUM") as pspool,
            tc.tile_pool(name="ocpool", bufs=4) as ocpool,
        ):
            # Warmup tile: N_WARMUP_MM dummy matmuls on zeroed SBUF ramp
            # the PE clock out of its low p-state while the first input chunk
            # and weights are still in flight.
            zt = wpool.tile([128, 2, SW], FP8)
            nc.vector.memset(zt[:], 0.0)

            # Weights land as the f32 high bytes (fp8 view): same sign, zero
            # blocks stay zero, and the gating DMA is 4x smaller.
            wblkf = wpool.tile([128, 6, 2, 128], FP8)
            sblk = wpool.tile([128, 6, 2, 128], FP8)
            wdr = [sblk[:, kw, :, :] for kw in range(3)]
            wb = [sblk[:, 3 + kw, :, :] for kw in range(3)]

            # scale[o] = mean(|w[o]|), O on partitions, duplicated on both
            # partition halves for the [128]-row eviction.
            w2 = wpool.tile([128, 576], F32)
            absw = wpool.tile([128, 576], F32)
            sc_sum = wpool.tile([128, 1], F32)
            sc128 = wpool.tile([128, 1], F32)

            # Resident sign plane; slot NV = V_0 copy.  Zero the pad columns
            # once (plane slots are written exactly once).
            assert n_batch == 2
            plane = wpool.tile([128, NV + 1, SW], FP8)
            nc.vector.memset(plane[:, :, 0:1], 0.0)
            nc.vector.memset(plane[:, :, 225:226], 0.0)
            nc.vector.memset(plane[:, :, 450:452], 0.0)

            def rhs(j, kw):
                return plane[:, j : j + 2, kw : kw + SN]

            def emit_sign(ic, c0, r0, k):
                """Sign rows r0..r0+k (absolute) from chunk starting at c0.
                ScalarE uses the Sign activation; GpSimd computes the same
                result bitwise on the raw bytes: (b & 0x80) | 0x38 is fp8
                +/-1.0 with the sign bit of b (x is never exactly 0)."""
                out_ap = plane[:, r0 : r0 + k, 1:451].rearrange(
                    "p j (b w) -> p j b w", w=225
                )[:, :, :, 0:w]
                in_ap = ic[:, r0 - c0 : r0 - c0 + k, :].rearrange(
                    "p j (b w) -> p j b w", b=n_batch
                )
                if DVE_SIGN0 and r0 < 8:
                    # First two batches ride VectorE so the first matmuls are
                    # not gated behind ScalarE's table load + weight sign.
                    nc.vector.tensor_scalar(
                        out_ap.bitcast(mybir.dt.uint8),
                        in_ap.bitcast(mybir.dt.uint8),
                        0x80, 0x38,
                        mybir.AluOpType.bitwise_and,
                        mybir.AluOpType.bitwise_or,
                    )
                else:
                    nc.scalar.sign(out_ap, in_ap)

            def evict(engine, ps, oc, oslot):
                out_ap = oc[:, oslot : oslot + GU, :].rearrange(
                    "p u (b w) -> p u b w", b=n_batch
                )
                in_ap = ps[:, 0:GU, 0:450].rearrange(
                    "p u (b w) -> p u b w", w=225
                )[:, :, :, 0:w]
                if engine == "act":
                    nc.scalar.mul(out_ap, in_ap, sc128[:])
                else:
                    nc.vector.tensor_scalar_mul(out_ap, in_ap, sc128[:])

            pm = (
                mybir.MatmulPerfMode.DoubleRowSwInterleave
                if SWI
                else mybir.MatmulPerfMode.DoubleRow
            )

            # ---- main pipeline ----
            next_chunk = 0   # next input chunk to DMA
            rows_avail = 0   # rows resident in SBUF (DMA emitted)
            rows_signed = 0  # rows signed into the plane
            ic = None
            ic_c0 = 0

            def ensure_signed(upto):
                """Emit chunk DMAs + sign ops until rows [0, upto) signed."""
                nonlocal next_chunk, rows_avail, rows_signed, ic, ic_c0
                while rows_signed < upto:
                    if rows_signed == rows_avail:
                        gc = sizes[next_chunk]
                        c0 = starts[next_chunk]
                        ic = icpool.tile([128, 16, n_batch * w], FP8, tag="ic")
                        nc.sync.dma_start(ic[:, 0:gc, :], xr[:, c0 : c0 + gc, :])
                        ic_c0 = c0
                        rows_avail += gc
                        next_chunk += 1
                    k = min(8, rows_avail - rows_signed)
                    emit_sign(ic, ic_c0, rows_signed, k)
                    rows_signed += k
                    if rows_signed - k == 0:
                        # V_0 copy for the boundary unit right after the
                        # first sign batch; then the weight-block sign (the
                        # first matmul's other gate) and the scale input.
                        nc.vector.tensor_copy(
                            out=plane[:, NV, 0:452], in_=plane[:, 0, 0:452]
                        )
                        nc.gpsimd.dma_start(
                            wblkf[:],
                            wblk.ap().rearrange("p (t i m) -> p t i m", t=6, i=2),
                        )
                        nc.scalar.sign(sblk[:], wblkf[:])
                        wr = wraw.ap().rearrange("(o f) one -> o (f one)", o=C)
                        nc.sync.dma_start(w2[0:64], wr)
                        nc.sync.dma_start(w2[64:128], wr)
                        # scale prep must be emitted before the first
                        # eviction (which reads sc128); it gates nothing
                        # earlier than that.
                        nc.scalar.activation(
                            out=absw[:], in_=w2[:],
                            func=mybir.ActivationFunctionType.Abs,
                            accum_out=sc_sum[:],
                        )
                        nc.scalar.mul(sc128[:], sc_sum[:], 1.0 / 576.0)

            # PE warmup: dep-free dummy matmuls on the zeroed tile.
            if N_WARMUP_MM:
                psw = pspool.tile([128, GU, 512], F32, tag="ps")
                for _ in range(N_WARMUP_MM):
                    nc.tensor.matmul(
                        psw[:, 0, 0:SN], zt[:, :, 0:128], zt[:, 0:2, 0:SN],
                        start=True, stop=True, perf_mode=pm,
                    )

            oc = None
            for g in range(n_groups):
                m0 = GU * g
                boundary = g == n_groups - 1
                nu = GU - 1 if boundary else GU  # interior units in group
                ensure_signed(min(m0 + nu + 1, NV))

                if m0 % OG == 0:
                    oc = ocpool.tile([128, OG, n_batch * w], F16, tag="oc")

                ps = pspool.tile([128, GU, 512], F32, tag="ps")
                for u in range(nu):
                    m = m0 + u
                    for kw in range(3):
                        nc.tensor.matmul(
                            ps[:, u, 0:SN], wdr[kw][:], rhs(m, kw),
                            start=(kw == 0), stop=(kw == 2),
                            perf_mode=pm,
                        )
                if boundary:
                    for kw in range(3):
                        nc.tensor.matmul(
                            ps[:, GU - 1, 0:SN], wb[kw][:], rhs(NV - 1, kw),
                            start=(kw == 0), stop=(kw == 2),
                            perf_mode=pm,
                        )

                evict("act" if g in ACT_EVICT_TAIL else "dve", ps, oc, m0 % OG)

                base = (m0 // OG) * OG
                if base == NV - OG:
                    # tail: store in halves so the last DMA is short
                    if m0 % OG == OG // 2 - GU:
                        nc.gpsimd.dma_start(
                            yr[:, base : base + OG // 2, :], oc[:, 0 : OG // 2, :]
                        )
                    elif boundary:
                        nc.gpsimd.dma_start(
                            yr[:, base + OG // 2 : base + OG, :],
                            oc[:, OG // 2 : OG, :],
                        )
                elif m0 % OG == OG - GU:
                    nc.gpsimd.dma_start(
                        yr[:, base : base + OG, :], oc[:, 0:OG, :]
                    )

    nc.compile()
    return nc


_NC_CACHE = {}


def _get_nc(key=(2,)):
    if key not in _NC_CACHE:
        _NC_CACHE[key] = build_nc(*key)
    return _NC_CACHE[key]


def _make_wblk(weights):
    """Arrange raw f32 weights into the 6-tile DoubleRow block layout
    [128, 6, 2, 128] (pure replication/zero-padding; sign runs on device)."""
    wt = weights.reshape(C, C, KH, KW).transpose(1, 2, 3, 0)  # [i, kh, kw, o]

    def T(kh, kw):
        return wt[:, kh, kw, :]  # W_{kh,kw}^T as [i, o]

    blk = np.zeros((128, 6, 2, 128), np.float32)
    for kw in range(KW):
        # interior tiles: i=0 -> [[W0, 0], [W1, W0]], i=1 -> [[W2, W1], [0, W2]]
        blk[0:64, kw, 0, 0:64] = T(0, kw)
        blk[64:128, kw, 0, 0:64] = T(1, kw)
        blk[64:128, kw, 0, 64:128] = T(0, kw)
        blk[0:64, kw, 1, 0:64] = T(2, kw)
        blk[0:64, kw, 1, 64:128] = T(1, kw)
        blk[64:128, kw, 1, 64:128] = T(2, kw)
        # boundary tiles: i=0 -> [[0, W0], [0, W1]], i=1 -> [[W1, 0], [W2, 0]]
        blk[0:64, 3 + kw, 0, 64:128] = T(0, kw)
        blk[64:128, 3 + kw, 0, 64:128] = T(1, kw)
        blk[0:64, 3 + kw, 1, 0:64] = T(1, kw)
        blk[64:128, 3 + kw, 1, 0:64] = T(2, kw)
    if SWI:
        # DoubleRowSwInterleave layout: per stationary tile, pairs (A, B)
        # interleaved per output column, columns reversed.
        swi = np.zeros((128, 6, 256), np.float32)
        swi[:, :, 0::2] = blk[:, :, 0, ::-1]
        swi[:, :, 1::2] = blk[:, :, 1, ::-1]
        blk = swi.reshape(128, 6, 2, 128)
    # ship only the f32 high byte (same sign; zeros stay 0x00 = fp8 zero)
    hb = blk.reshape(128, 6 * 2 * 128, 1).view(np.uint8)[:, :, 3]
    return np.ascontiguousarray(hb).view(ml_dtypes.float8_e4m3fn)


def pack_x(x_shard):
    """f32 [nb, C, h, w] -> high-byte plane [128, NV, nb, w] (fp8e4 view);
    p = parity*64 + channel.  Pure strided relayout of the sign/exponent
    byte -- no arithmetic."""
    nb = x_shard.shape[0]
    hb = x_shard.view(np.uint8).reshape(nb, C, NV, 2, W, 4)[..., 3]
    packed = np.ascontiguousarray(hb.transpose(3, 1, 2, 0, 4)).reshape(
        128, NV, nb, W
    )
    return packed.view(ml_dtypes.float8_e4m3fn)


def unpack_y(ypk):
    """fp16 [128, NV, nb, w] -> f32 [nb, C, h, w] per the unit layout."""
    nb = ypk.shape[2]
    y = np.empty((nb, C, H, W), np.float32)
    # interior units m=0..NV-2 -> rows 2m+1 (p<64) and 2m+2 (p>=64)
    y[:, :, 1 : H - 1 : 2] = ypk[0:C, 0 : NV - 1].transpose(2, 0, 1, 3)
    y[:, :, 2 : H : 2] = ypk[C:128, 0 : NV - 1].transpose(2, 0, 1, 3)
    # boundary unit: p<64 -> row 0, p>=64 -> row H-1
    y[:, :, 0] = ypk[0:C, NV - 1].transpose(1, 0, 2)
    y[:, :, H - 1] = ypk[C:128, NV - 1].transpose(1, 0, 2)
    return y


def make_in_maps(x, weights):
    x = np.ascontiguousarray(np.asarray(x, dtype=np.float32))
    weights = np.asarray(weights, dtype=np.float32)
    wblk = _make_wblk(weights)
    nb = FULL_BATCH // N_CORES
    return [
        {
            "xb": pack_x(x[c * nb : (c + 1) * nb]),
            "wraw": weights,
            "wblk": wblk,
        }
        for c in range(N_CORES)
    ]


def gather_out(results):
    return np.concatenate([unpack_y(r["yb"]) for r in results], axis=0)


def kernel(x, weights):
    from concourse import bass_utils

    nc = _get_nc()
    in_maps = make_in_maps(x, weights)
    res = bass_utils.run_bass_kernel_spmd(nc, in_maps, core_ids=list(range(N_CORES)))
    return gather_out(res.results)


# revision 21
# speedup vs baseline: 1.0265x; 1.0001x over previous
"""Binary conv forward kernel for Trainium2 (8 NeuronCores, data-parallel over batch).

Computes y = conv2d(sign(x), scale[o] * sign(w)), stride 1, pad 1, NCHW/OIHW,
x [16, 64, 224, 224] f32, w [64*64*3*3, 1] f32 -> y [16, 64, 224, 224] f32.

Sharding: batch 16 -> 2 images per core, weights replicated (tiny).

HBM traffic is the roofline, so both directions are compressed:
  - Input: the host ships only the HIGH BYTE of each f32 (pure strided
    relayout, no arithmetic), packed [128, pair, batch, w].  Reinterpreted as
    fp8e4, that byte has the same sign as x (sign bit + top-7 exponent bits),
    so ScalarE's sign() recovers sign(x) exactly -- randn never produces
    |x| < 2^-126 (byte 0x00/0x80) or |x| >= 2^127 (NaN bytes).  4x fewer
    input bytes.
  - Output: written as fp16.  y = scale[o] * S with S an integer in
    [-576, 576]; fp16 rounding error is <= 2^-11 relative, far inside the
    2e-2 gate.  2x fewer output bytes.  Host upcasts to f32 on unpack.

Device algorithm (per core, n_batch=2 images):
  - A resident fp8 sign plane [128, NV+1, 464]: slot j = image rows (2j, 2j+1)
    (even row on partitions 0..63, odd on 64..127), both batch images in the
    free dim with one shared zero column between and zero pads at the edges so
    the kw shifts are exact.  Slot NV = copy of slot 0 (for the row-0/223
    boundary unit).  ScalarE signs 8 rows per ACTIVATE to amortize overhead.
  - Interior output pair (2m+1, 2m+2) accumulates in one PSUM bank via 3
    DoubleRow matmuls (virtual K=256 over slots m, m+1; M=128; N=450), one per
    kw shift.  Stationary blocks [[W0,0],[W1,W0]] / [[W2,W1],[0,W2]], where
    Wk = sign(w)[:,:,kh=k,kw]^T.  Boundary rows 0 and 223 use slots NV-1 and
    NV with blocks [[0,W0],[0,W1]] / [[W1,0],[W2,0]].
  - PSUM tiles span 4 banks = 4 units; eviction multiplies by the
    per-partition scale[o] (computed on device from raw weights) and writes
    fp16: one VectorE op per 4 units (last 3 groups ride ScalarE to balance
    engine load).
  - Input DMAs ride the HWDGE ring (nc.sync); weight + output DMAs ride SWDGE
    (nc.gpsimd) so loads and stores overlap on HBM.
"""

import numpy as np
import ml_dtypes

import concourse.bacc as bacc
import concourse.mybir as mybir
import concourse.tile as tile

F32 = mybir.dt.float32
F16 = mybir.dt.float16
FP8 = mybir.dt.float8e4

N_CORES = 8
FULL_BATCH = 16
C = 64  # in channels == out channels
H = 224
W = 224
KH = KW = 3
NV = H // 2  # row-pair slots / output units per image pair
# Sign-plane slot layout (fp8, per partition): [pad, b0 w=224, sep, b1 w=224,
# pad, pad] = 452 cols used, padded to 464 (multiple of 16 for DoubleRow AP
# steps).
SW = 464   # slot stride
SN = 449   # matmul N (448 real output columns + 1 junk)

GU = 2           # units per PSUM tile (2 banks; bufs=4 -> PE runs 3 groups ahead)
OG = 8           # units per output chunk
ACT_EVICT_TAIL = (53, 55)  # post-sign-stream evictions moved to ScalarE
DVE_SIGN0 = True  # first two sign batches on VectorE (bitwise)
N_WARMUP_MM = 13  # dummy matmuls on a zeroed tile bridge engine-init to the first sign
SWI = False


def build_nc(n_batch=2):
    """Build the single-core Bass module (same NEFF runs on all 8 cores)."""
    nc = bacc.Bacc("TRN2", target_bir_lowering=False, debug=False)

    w = W
    xb = nc.dram_tensor("xb", [128, NV, n_batch, w], FP8, kind="ExternalInput")
    wraw = nc.dram_tensor("wraw", [C * C * KH * KW, 1], F32, kind="ExternalInput")
    # wblk: host-arranged raw f32 weights in the 6-tile DoubleRow block layout
    # [128, (3 interior + 3 boundary) tiles, 2, 128] with zeros in the zero
    # blocks (pure replication/padding; sign runs on device, sign(0) = 0).
    wblk = nc.dram_tensor("wblk", [128, 6 * 2 * 128], FP8, kind="ExternalInput")
    yb = nc.dram_tensor("yb", [128, NV, n_batch, w], F16, kind="ExternalOutput")

    xr = xb.ap().rearrange("p j b w -> p j (b w)")   # [128, NV, 448]
    yr = yb.ap().rearrange("p j b w -> p j (b w)")

    # Input chunk schedule: small leading chunks so signing (and the first
    # matmuls) start as early as possible, then full chunks.
    sizes = [4, 4, 8] + [16] * 6
    assert sum(sizes) == NV
    starts = list(np.cumsum([0] + sizes[:-1]))

    # PSUM groups: 55 x 2 interior units, then [110, boundary].
    n_groups = 56

    with tile.TileContext(nc) as tc:
        with (
            tc.tile_pool(name="wpool", bufs=1) as wpool,
            tc.tile_pool(name="icpool", bufs=4) as icpool,
            tc.tile_pool(name="pspool", bufs=4, space="PSUM") as pspool,
            tc.tile_pool(name="ocpool", bufs=4) as ocpool,
        ):
            # Warmup tile: N_WARMUP_MM dummy matmuls on zeroed SBUF ramp
            # the PE clock out of its low p-state while the first input chunk
            # and weights are still in flight.
            zt = wpool.tile([128, 2, SW], FP8)
            nc.vector.memset(zt[:], 0.0)

            # Weights land as the f32 high bytes (fp8 view): same sign, zero
            # blocks stay zero, and the gating DMA is 4x smaller.
            wblkf = wpool.tile([128, 6, 2, 128], FP8)
            sblk = wpool.tile([128, 6, 2, 128], FP8)
            wdr = [sblk[:, kw, :, :] for kw in range(3)]
            wb = [sblk[:, 3 + kw, :, :] for kw in range(3)]

            # scale[o] = mean(|w[o]|), O on partitions, duplicated on both
            # partition halves for the [128]-row eviction.
            w2 = wpool.tile([128, 576], F32)
            absw = wpool.tile([128, 576], F32)
            sc_sum = wpool.tile([128, 1], F32)
            sc128 = wpool.tile([128, 1], F32)

            # Resident sign plane; slot NV = V_0 copy.  Zero the pad columns
            # once (plane slots are written exactly once).
            assert n_batch == 2
            plane = wpool.tile([128, NV + 1, SW], FP8)
            nc.vector.memset(plane[:, :, 0:1], 0.0)
            nc.vector.memset(plane[:, :, 225:226], 0.0)
            nc.vector.memset(plane[:, :, 450:452], 0.0)

            def rhs(j, kw):
                return plane[:, j : j + 2, kw : kw + SN]

            def emit_sign(ic, c0, r0, k):
                """Sign rows r0..r0+k (absolute) from chunk starting at c0.
                ScalarE uses the Sign activation; GpSimd computes the same
                result bitwise on the raw bytes: (b & 0x80) | 0x38 is fp8
                +/-1.0 with the sign bit of b (x is never exactly 0)."""
                out_ap = plane[:, r0 : r0 + k, 1:451].rearrange(
                    "p j (b w) -> p j b w", w=225
                )[:, :, :, 0:w]
                in_ap = ic[:, r0 - c0 : r0 - c0 + k, :].rearrange(
                    "p j (b w) -> p j b w", b=n_batch
                )
                if DVE_SIGN0 and r0 < 8:
                    # First two batches ride VectorE so the first matmuls are
                    # not gated behind ScalarE's table load + weight sign.
                    nc.vector.tensor_scalar(
                        out_ap.bitcast(mybir.dt.uint8),
                        in_ap.bitcast(mybir.dt.uint8),
                        0x80, 0x38,
                        mybir.AluOpType.bitwise_and,
                        mybir.AluOpType.bitwise_or,
                    )
                else:
                    nc.scalar.sign(out_ap, in_ap)

            def evict(engine, ps, oc, oslot):
                out_ap = oc[:, oslot : oslot + GU, :].rearrange(
                    "p u (b w) -> p u b w", b=n_batch
                )
                in_ap = ps[:, 0:GU, 0:450].rearrange(
                    "p u (b w) -> p u b w", w=225
                )[:, :, :, 0:w]
                if engine == "act":
                    nc.scalar.mul(out_ap, in_ap, sc128[:])
                else:
                    nc.vector.tensor_scalar_mul(out_ap, in_ap, sc128[:])

            pm = (
                mybir.MatmulPerfMode.DoubleRowSwInterleave
                if SWI
                else mybir.MatmulPerfMode.DoubleRow
            )

            # ---- main pipeline ----
            next_chunk = 0   # next input chunk to DMA
            rows_avail = 0   # rows resident in SBUF (DMA emitted)
            rows_signed = 0  # rows signed into the plane
            ic = None
            ic_c0 = 0

            def ensure_signed(upto):
                """Emit chunk DMAs + sign ops until rows [0, upto) signed."""
                nonlocal next_chunk, rows_avail, rows_signed, ic, ic_c0
                while rows_signed < upto:
                    if rows_signed == rows_avail:
                        gc = sizes[next_chunk]
                        c0 = starts[next_chunk]
                        ic = icpool.tile([128, 16, n_batch * w], FP8, tag="ic")
                        nc.sync.dma_start(ic[:, 0:gc, :], xr[:, c0 : c0 + gc, :])
                        ic_c0 = c0
                        rows_avail += gc
                        next_chunk += 1
                    k = min(8, rows_avail - rows_signed)
                    if rows_signed == 0:
                        k = 2  # tiny first batch opens group 0 earlier
                    emit_sign(ic, ic_c0, rows_signed, k)
                    rows_signed += k
                    if rows_signed - k == 0:
                        # V_0 copy for the boundary unit right after the
                        # first sign batch; then the weight-block sign (the
                        # first matmul's other gate) and the scale input.
                        nc.vector.tensor_copy(
                            out=plane[:, NV, 0:452], in_=plane[:, 0, 0:452]
                        )
                        nc.gpsimd.dma_start(
                            wblkf[:],
                            wblk.ap().rearrange("p (t i m) -> p t i m", t=6, i=2),
                        )
                        nc.scalar.sign(sblk[:], wblkf[:])
                        wr = wraw.ap().rearrange("(o f) one -> o (f one)", o=C)
                        nc.sync.dma_start(w2[0:64], wr)
                        nc.sync.dma_start(w2[64:128], wr)
                        # scale prep must be emitted before the first
                        # eviction (which reads sc128); it gates nothing
                        # earlier than that.
                        nc.scalar.activation(
                            out=absw[:], in_=w2[:],
                            func=mybir.ActivationFunctionType.Abs,
                            accum_out=sc_sum[:],
                        )
                        nc.scalar.mul(sc128[:], sc_sum[:], 1.0 / 576.0)

            # PE warmup: dep-free dummy matmuls on the zeroed tile.
            if N_WARMUP_MM:
                psw = pspool.tile([128, GU, 512], F32, tag="ps")
                for _ in range(N_WARMUP_MM):
                    nc.tensor.matmul(
                        psw[:, 0, 0:SN], zt[:, :, 0:128], zt[:, 0:2, 0:SN],
                        start=True, stop=True, perf_mode=pm,
                    )

            oc = None
            for g in range(n_groups):
                m0 = GU * g
                boundary = g == n_groups - 1
                nu = GU - 1 if boundary else GU  # interior units in group
                ensure_signed(min(m0 + nu + 1, NV))

                if m0 % OG == 0:
                    oc = ocpool.tile([128, OG, n_batch * w], F16, tag="oc")

                ps = pspool.tile([128, GU, 512], F32, tag="ps")
                for u in range(nu):
                    m = m0 + u
                    for kw in range(3):
                        nc.tensor.matmul(
                            ps[:, u, 0:SN], wdr[kw][:], rhs(m, kw),
                            start=(kw == 0), stop=(kw == 2),
                            perf_mode=pm,
                        )
                if boundary:
                    for kw in range(3):
                        nc.tensor.matmul(
                            ps[:, GU - 1, 0:SN], wb[kw][:], rhs(NV - 1, kw),
                            start=(kw == 0), stop=(kw == 2),
                            perf_mode=pm,
                        )

                evict("act" if g in ACT_EVICT_TAIL else "dve", ps, oc, m0 % OG)

                base = (m0 // OG) * OG
                if base == NV - OG:
                    # tail: store in halves so the last DMA is short
                    if m0 % OG == OG // 2 - GU:
                        nc.gpsimd.dma_start(
                            yr[:, base : base + OG // 2, :], oc[:, 0 : OG // 2, :]
                        )
                    elif boundary:
                        nc.gpsimd.dma_start(
                            yr[:, base + OG // 2 : base + OG, :],
                            oc[:, OG // 2 : OG, :],
                        )
                elif m0 % OG == OG - GU:
                    nc.gpsimd.dma_start(
                        yr[:, base : base + OG, :], oc[:, 0:OG, :]
                    )

    nc.compile()
    return nc


_NC_CACHE = {}


def _get_nc(key=(2,)):
    if key not in _NC_CACHE:
        _NC_CACHE[key] = build_nc(*key)
    return _NC_CACHE[key]


def _make_wblk(weights):
    """Arrange raw f32 weights into the 6-tile DoubleRow block layout
    [128, 6, 2, 128] (pure replication/zero-padding; sign runs on device)."""
    wt = weights.reshape(C, C, KH, KW).transpose(1, 2, 3, 0)  # [i, kh, kw, o]

    def T(kh, kw):
        return wt[:, kh, kw, :]  # W_{kh,kw}^T as [i, o]

    blk = np.zeros((128, 6, 2, 128), np.float32)
    for kw in range(KW):
        # interior tiles: i=0 -> [[W0, 0], [W1, W0]], i=1 -> [[W2, W1], [0, W2]]
        blk[0:64, kw, 0, 0:64] = T(0, kw)
        blk[64:128, kw, 0, 0:64] = T(1, kw)
        blk[64:128, kw, 0, 64:128] = T(0, kw)
        blk[0:64, kw, 1, 0:64] = T(2, kw)
        blk[0:64, kw, 1, 64:128] = T(1, kw)
        blk[64:128, kw, 1, 64:128] = T(2, kw)
        # boundary tiles: i=0 -> [[0, W0], [0, W1]], i=1 -> [[W1, 0], [W2, 0]]
        blk[0:64, 3 + kw, 0, 64:128] = T(0, kw)
        blk[64:128, 3 + kw, 0, 64:128] = T(1, kw)
        blk[0:64, 3 + kw, 1, 0:64] = T(1, kw)
        blk[64:128, 3 + kw, 1, 0:64] = T(2, kw)
    if SWI:
        # DoubleRowSwInterleave layout: per stationary tile, pairs (A, B)
        # interleaved per output column, columns reversed.
        swi = np.zeros((128, 6, 256), np.float32)
        swi[:, :, 0::2] = blk[:, :, 0, ::-1]
        swi[:, :, 1::2] = blk[:, :, 1, ::-1]
        blk = swi.reshape(128, 6, 2, 128)
    # ship only the f32 high byte (same sign; zeros stay 0x00 = fp8 zero)
    hb = blk.reshape(128, 6 * 2 * 128, 1).view(np.uint8)[:, :, 3]
    return np.ascontiguousarray(hb).view(ml_dtypes.float8_e4m3fn)


def pack_x(x_shard):
    """f32 [nb, C, h, w] -> high-byte plane [128, NV, nb, w] (fp8e4 view);
    p = parity*64 + channel.  Pure strided relayout of the sign/exponent
    byte -- no arithmetic."""
    nb = x_shard.shape[0]
    hb = x_shard.view(np.uint8).reshape(nb, C, NV, 2, W, 4)[..., 3]
    packed = np.ascontiguousarray(hb.transpose(3, 1, 2, 0, 4)).reshape(
        128, NV, nb, W
    )
    return packed.view(ml_dtypes.float8_e4m3fn)


def unpack_y(ypk):
    """fp16 [128, NV, nb, w] -> f32 [nb, C, h, w] per the unit layout."""
    nb = ypk.shape[2]
    y = np.empty((nb, C, H, W), np.float32)
    # interior units m=0..NV-2 -> rows 2m+1 (p<64) and 2m+2 (p>=64)
    y[:, :, 1 : H - 1 : 2] = ypk[0:C, 0 : NV - 1].transpose(2, 0, 1, 3)
    y[:, :, 2 : H : 2] = ypk[C:128, 0 : NV - 1].transpose(2, 0, 1, 3)
    # boundary unit: p<64 -> row 0, p>=64 -> row H-1
    y[:, :, 0] = ypk[0:C, NV - 1].transpose(1, 0, 2)
    y[:, :, H - 1] = ypk[C:128, NV - 1].transpose(1, 0, 2)
    return y


def make_in_maps(x, weights):
    x = np.ascontiguousarray(np.asarray(x, dtype=np.float32))
    weights = np.asarray(weights, dtype=np.float32)
    wblk = _make_wblk(weights)
    nb = FULL_BATCH // N_CORES
    return [
        {
            "xb": pack_x(x[c * nb : (c + 1) * nb]),
            "wraw": weights,
            "wblk": wblk,
        }
        for c in range(N_CORES)
    ]


def gather_out(results):
    return np.concatenate([unpack_y(r["yb"]) for r in results], axis=0)


def kernel(x, weights):
    from concourse import bass_utils

    nc = _get_nc()
    in_maps = make_in_maps(x, weights)
    res = bass_utils.run_bass_kernel_spmd(nc, in_maps, core_ids=list(range(N_CORES)))
    return gather_out(res.results)
